# revision 1
# baseline (speedup 1.0000x reference)
"""GQA attention kernel for Trainium2, 8 NeuronCores.

Sharding: data-parallel over batch (B=2) x 4 head-shards -> 8 cores.
Shard s owns q-heads {2s, 2s+1, 2s+8, 2s+9}; heads h and h+8 are
rotate-half RoPE partners, so each shard's RoPE is self-contained (no
duplicated Q projection).  Those 4 heads use kv-heads {s//2, s//2+2},
which are themselves RoPE partners on the K side.  out_proj is
row-parallel; partials are summed on host.

Pipeline:
  front: consolidated DMAs -> K/V/Q projections (PSUM accumulate over
    hidden chunks) -> RoPE (DVE) -> rmsnorm via block-ones matmul
    (PE broadcast) + Square/Sqrt (ACT) + reciprocal (DVE).
  attention (flat software pipeline over 128 (ic, jc) steps):
    scoresT[j, i] matmul -> exp (ACT, the bottleneck engine) ->
    PV in natural layout (lhsT = probs, rhs = v||ones so the softmax
    denominator accumulates in column 64) with 2-step lookahead ->
    per-ic: reciprocal-normalize (per-partition scalar, DVE),
    o_nat -> oT via XBAR dma transpose, out_proj matmul, y copy.
"""

import numpy as np
import ml_dtypes

import concourse.bacc as bacc
import concourse.mybir as mybir
from concourse.tile import TileContext
from concourse.bass_utils import run_bass_kernel_spmd

BF16 = mybir.dt.bfloat16
F32 = mybir.dt.float32
I16 = mybir.dt.int16
AL = mybir.AluOpType
AF = mybir.ActivationFunctionType

# Schraudolph fast-exp in bf16 bit-space: exp(s/8) ~= bitcast_bf16(
# int16(SCH_A*s + SCH_B)).  One DVE tensor_scalar writing the probability
# tile's int16 view - offloads part of the softmax exp from the saturated
# ACT engine.  RMS rel err ~1.8% on N(0,1) scores.
SCH_A = (2.0 ** 7 / np.log(2.0)) * 0.125
SCH_B = 127.0 * 2.0 ** 7 - 7.375

B, S, HID = 2, 2048, 1024
H, HKV, D = 16, 4, 64
ROPE_BASE = 10000.0
EPS = float(np.finfo(np.float32).eps)
NB = ml_dtypes.bfloat16

NIC = 8          # i-chunks of 256
ICS = S // NIC   # 256
NJC = 16         # j-chunks of 128

# per-jc exp engine within each i-chunk: most on ACT (exact), some via
# Schraudolph on DVE to relieve the ACT bottleneck
JC_ENG = ["act"] * NJC
for _j in (1, 3, 5, 7, 9, 11, 13, 14):
    JC_ENG[_j] = "dve"

_cache: dict = {}


def _build(use_mask: bool, use_bias: bool):
    nc = bacc.Bacc("TRN2", target_bir_lowering=False)

    hT = nc.dram_tensor("hT", [128, 8, S], BF16, kind="ExternalInput")
    wq = nc.dram_tensor("wq", [128, 8, 256], BF16, kind="ExternalInput")
    wkv = nc.dram_tensor("wkv", [128, 8, 256], BF16, kind="ExternalInput")
    wo = nc.dram_tensor("wo", [128, 2, HID], BF16, kind="ExternalInput")
    qtab = nc.dram_tensor("qtab", [128, 2, S], BF16, kind="ExternalInput")
    ktab = nc.dram_tensor("ktab", [64, 2, S], BF16, kind="ExternalInput")
    if use_bias:
        bias = nc.dram_tensor("bias", [1, 512], F32, kind="ExternalInput")
    mk = (
        nc.dram_tensor("mk", [NJC, 128, S], F32, kind="ExternalInput")
        if use_mask
        else None
    )
    y = nc.dram_tensor("y", [128, NIC, 2, HID], BF16, kind="ExternalOutput")

    with TileContext(nc) as tc:
        with tc.tile_pool(name="const", bufs=1) as cp:
            # ---- persistent SBUF tiles --------------------------------
            hT_sb = cp.tile([128, 8, S], BF16)
            wq_sb = cp.tile([128, 8, 256], BF16)
            wkv_sb = cp.tile([128, 8, 256], BF16)  # [:, :, 0:128]=wk, 128:256=wv
            wo_sb = cp.tile([128, 2, HID], BF16)
            qco_sb = cp.tile([128, S], BF16)
            qsi_sb = cp.tile([128, S], BF16)
            kco_sb = cp.tile([64, S], BF16)
            ksi_sb = cp.tile([64, S], BF16)

            qn = cp.tile([128, 2, S], BF16)    # rmsnorm'd roped q
            kn_a2 = cp.tile([128, S], BF16)    # normalized kv_a, duplicated rows
            kn_b2 = cp.tile([128, S], BF16)
            v_all = cp.tile([128, NJC, 2, 65], BF16)  # v natural + ones col
            y_sb = cp.tile([128, NIC, 2, HID], BF16)
            oT = cp.tile([128, 2, NIC, ICS], BF16)  # [d-part, cc, ic, i]

            eps_sb = cp.tile([128, 1], F32)
            nc.vector.memset(eps_sb[:], EPS)
            onesq = cp.tile([128, 128], BF16)  # block-diag 1/64
            nc.vector.memset(onesq[:], 0.0)
            nc.vector.memset(onesq[0:64, 0:64], 1.0 / 64.0)
            nc.vector.memset(onesq[64:128, 64:128], 1.0 / 64.0)
            onesk = cp.tile([64, 64], BF16)
            nc.vector.memset(onesk[:], 1.0 / 64.0)
            nc.vector.memset(v_all[:], 1.0)
            if use_bias:
                ones_row = cp.tile([1, 512], BF16)
                nc.vector.memset(ones_row[:], 1.0)
                bias_sb = cp.tile([1, 512], F32)

            # ---- input DMAs, ordered for earliest compute start -------
            nc.sync.dma_start(out=wkv_sb[:], in_=wkv[:])
            for hc in range(4):
                nc.sync.dma_start(
                    out=hT_sb[:, 2 * hc:2 * hc + 2, :], in_=hT[:, 2 * hc:2 * hc + 2, :]
                )
            nc.sync.dma_start(out=kco_sb[:], in_=ktab[:, 0, :])
            nc.sync.dma_start(out=ksi_sb[:], in_=ktab[:, 1, :])
            nc.sync.dma_start(out=wq_sb[:], in_=wq[:])
            nc.sync.dma_start(out=qco_sb[:], in_=qtab[:, 0, :])
            nc.sync.dma_start(out=qsi_sb[:], in_=qtab[:, 1, :])
            nc.sync.dma_start(out=wo_sb[:], in_=wo[:])
            if use_bias:
                nc.sync.dma_start(out=bias_sb[:], in_=bias[:])

            # ---- front phase: projections + rope + rmsnorm ------------
            # Software-pipelined per 512-col so-chunk: PE runs dense
            # projection streams; the rms matmuls of chunk so-1 are emitted
            # after chunk so's projections so their ACT/DVE input chain has
            # already drained.  Rope combines and the final normalize
            # multiplies run on the otherwise-idle Pool engine.
            with (
                tc.tile_pool(name="fsb", bufs=2) as fsb,
                tc.tile_pool(name="fps", bufs=1, space="PSUM") as fp,
            ):
                def emit_proj(so):
                    sl = slice(so * 512, (so + 1) * 512)
                    psk = fp.tile([128, 2, 512], F32, tag="big", bufs=3,
                                  name="psk")[0:64]
                    for t in range(2):
                        for ko in range(8):
                            nc.tensor.matmul(
                                psk[:, t, :],
                                lhsT=wkv_sb[:, ko, t * 64:(t + 1) * 64],
                                rhs=hT_sb[:, ko, sl],
                                start=(ko == 0),
                                stop=(ko == 7) if not use_bias else False,
                            )
                        if use_bias:
                            nc.tensor.matmul(
                                psk[:, t, :],
                                lhsT=bias_sb[:, 256 + t * 64:256 + t * 64 + 64],
                                rhs=ones_row[:],
                                start=False, stop=True,
                            )
                    # V projection for 4 position-chunks, natural layout:
                    # dense PE filler while the k-rope chain drains psk
                    for sc in range(4 * so, 4 * so + 4):
                        psv = fp.tile([128, 128], F32, tag="psv", bufs=2,
                                      name="psv")
                        for ko in range(8):
                            nc.tensor.matmul(
                                psv[:],
                                lhsT=hT_sb[:, ko, sc * 128:(sc + 1) * 128],
                                rhs=wkv_sb[:, ko, 128:256],
                                start=(ko == 0),
                                stop=(ko == 7) if not use_bias else False,
                            )
                        if use_bias:
                            nc.tensor.matmul(
                                psv[:],
                                lhsT=ones_row[:, 0:128],
                                rhs=bias_sb[:, 384:512],
                                start=False, stop=True,
                            )
                        nc.vector.tensor_copy(
                            v_all[:, sc, :, 0:64],
                            psv[:].rearrange("p (s d) -> p s d", s=2),
                        )
                    psq = fp.tile([128, 2, 512], F32, tag="big", bufs=3,
                                  name="psq")
                    for ch in range(2):
                        for ko in range(8):
                            nc.tensor.matmul(
                                psq[:, ch, :],
                                lhsT=wq_sb[:, ko, ch * 128:(ch + 1) * 128],
                                rhs=hT_sb[:, ko, sl],
                                start=(ko == 0),
                                stop=(ko == 7) if not use_bias else False,
                            )
                        if use_bias:
                            nc.tensor.matmul(
                                psq[:, ch, :],
                                lhsT=bias_sb[:, ch * 128:(ch + 1) * 128],
                                rhs=ones_row[:],
                                start=False, stop=True,
                            )
                    return psk, psq

                def emit_rope(so, raw, pfx, co, si, npart):
                    """ACT casts the raw projection to bf16 so every DVE rope
                    op runs in 2x 16-bit mode; ACT squares for the rms."""
                    sl = slice(so * 512, (so + 1) * 512)
                    rawb = fsb.tile([128, 2, 512], BF16, tag=pfx + "rb",
                                    name="rb")[:npart]
                    nc.scalar.copy(rawb, raw)
                    tcos = fsb.tile([128, 2, 512], BF16, tag=pfx + "tc",
                                    name="tc")[:npart]
                    tsin = fsb.tile([128, 2, 512], BF16, tag=pfx + "ts",
                                    name="ts")[:npart]
                    nc.vector.tensor_tensor(
                        tcos, rawb, co[:, None, sl].to_broadcast((npart, 2, 512)),
                        AL.mult,
                    )
                    nc.vector.tensor_tensor(
                        tsin, rawb, si[:, None, sl].to_broadcast((npart, 2, 512)),
                        AL.mult,
                    )
                    rp = fsb.tile([128, 2, 512], BF16, tag=pfx + "rp",
                                  name="rp")[:npart]
                    nc.vector.tensor_tensor(
                        rp[:, 0, :], tcos[:, 0, :], tsin[:, 1, :], AL.subtract
                    )
                    nc.vector.tensor_tensor(
                        rp[:, 1, :], tcos[:, 1, :], tsin[:, 0, :], AL.add
                    )
                    sq = fsb.tile([128, 2, 512], BF16, tag=pfx + "sq",
                                  name="sq")[:npart]
                    nc.scalar.activation(sq, rp, AF.Square)
                    return rp, sq

                def emit_rms(so, rp, sq, npart, rms_lhs, out0, out1, dup):
                    """Block-ones matmul (mean+broadcast), sqrt, recip, then
                    normalize multiplies on Pool; optional row-dup DMAs."""
                    sl = slice(so * 512, (so + 1) * 512)
                    psr = fp.tile([128, 2, 512], F32, tag="big", bufs=3,
                                  name="psr")[:npart]
                    for ch in range(2):
                        nc.tensor.matmul(
                            psr[:, ch, :], lhsT=rms_lhs, rhs=sq[:, ch, :],
                            start=True, stop=True,
                        )
                    rs = fsb.tile([128, 2, 512], F32, tag="rs" + str(npart),
                                  name="rs")[:npart]
                    nc.scalar.activation(rs, psr, AF.Sqrt, bias=eps_sb[:npart])
                    nc.vector.reciprocal(rs, rs)
                    nc.vector.tensor_tensor(
                        out0[:npart, sl], rp[:, 0, :], rs[:, 0, :], AL.mult
                    )
                    nc.gpsimd.tensor_tensor(
                        out1[:npart, sl], rp[:, 1, :], rs[:, 1, :], AL.mult
                    )
                    if dup:
                        # duplicate normalized k rows so scores lhsT partition
                        # base matches either q-half
                        nc.sync.dma_start(out=out0[64:128, sl], in_=out0[0:64, sl])
                        nc.sync.dma_start(out=out1[64:128, sl], in_=out1[0:64, sl])

                pend = None
                for so in range(4):
                    psk, psq = emit_proj(so)
                    rpk, sqk = emit_rope(so, psk, "k", kco_sb, ksi_sb, 64)
                    rpq, sqq = emit_rope(so, psq, "q", qco_sb, qsi_sb, 128)
                    if pend is not None:
                        emit_rms(so - 1, *pend[0])
                        emit_rms(so - 1, *pend[1])
                    pend = (
                        (rpk, sqk, 64, onesk, kn_a2, kn_b2, True),
                        (rpq, sqq, 128, onesq, qn[:, 0, :], qn[:, 1, :], False),
                    )
                emit_rms(3, *pend[1])
                emit_rms(3, *pend[0])

            # prewarm the exp activation table while front drains
            warm = cp.tile([1, 1], F32)
            nc.scalar.activation(warm[:], eps_sb[0:1, :], AF.Exp)

            # ---- attention: flat pipeline over 128 (ic, jc) steps -----
            with (
                tc.tile_pool(name="asb", bufs=1) as ab,
                tc.tile_pool(name="aps", bufs=1, space="PSUM") as ap,
            ):
                pso: dict = {}
                pT: dict = {}

                def scores_exp(g):
                    ic, jc = divmod(g, NJC)
                    isl = slice(ic * ICS, (ic + 1) * ICS)
                    pss = ap.tile([128, 4, ICS], F32, tag="pss", bufs=3, name="pss")
                    # slice order hs = 2*half + ch so the two slices sharing
                    # each 2KB psum zero region use the same operand partition
                    # base (mid-group base changes break the NEFF path)
                    for hs in range(4):
                        half, ch = divmod(hs, 2)
                        knt = kn_a2 if ch == 0 else kn_b2
                        qrows = slice(half * 64, half * 64 + 64)
                        nc.tensor.matmul(
                            pss[:, hs, :],
                            lhsT=knt[qrows, jc * 128:(jc + 1) * 128],
                            rhs=qn[qrows, ch, isl],
                            start=(ch == 0), stop=(ch == 1),
                        )
                    pt = ab.tile([128, 4, ICS], BF16, tag="pT", bufs=5, name="pt")
                    if use_mask:
                        mkt = ab.tile([128, ICS], F32, tag="mkt", bufs=2, name="mkt")
                        nc.sync.dma_start(out=mkt[:], in_=mk[jc][:, isl])
                        sm = ab.tile([128, 4, ICS], F32, tag="sm", bufs=2, name="sm")
                        nc.vector.scalar_tensor_tensor(
                            sm, pss[:], 0.125,
                            mkt[:, None, :].to_broadcast((128, 4, ICS)),
                            AL.mult, AL.add,
                        )
                        nc.scalar.activation(pt, sm, AF.Exp)
                    elif JC_ENG[jc] == "act":
                        nc.scalar.activation(pt, pss, AF.Exp, scale=0.125)
                    else:
                        nc.vector.tensor_scalar(
                            pt.bitcast(I16), pss[:], SCH_A, SCH_B, AL.mult, AL.add
                        )
                    pT[g] = pt

                def pv(g):
                    ic, jc = divmod(g, NJC)
                    if jc == 0:
                        # one bank holds both isubs' PV accumulators; the
                        # softmax denominators accumulate in a second bank
                        pso[ic] = (
                            ap.tile([128, 2, 4, 64], F32, tag="pso", bufs=1,
                                    name="pso"),
                            ap.tile([128, 2, 4, 1], F32, tag="den", bufs=1,
                                    name="den"),
                        )
                    po, de = pso[ic]
                    pt = pT.pop(g)
                    for isub in range(2):
                        for hs in range(4):
                            first = jc == 0 and isub == 0 and hs == 0
                            last = jc == NJC - 1 and isub == 1 and hs == 3
                            nc.tensor.matmul(
                                po[:, isub, hs, :],
                                lhsT=pt[:, hs, isub * 128:(isub + 1) * 128],
                                rhs=v_all[:, jc, hs % 2, 0:64],
                                start=first, stop=last,
                            )
                            nc.tensor.matmul(
                                de[:, isub, hs, :],
                                lhsT=pt[:, hs, isub * 128:(isub + 1) * 128],
                                rhs=v_all[:, jc, hs % 2, 64:65],
                                start=first, stop=last,
                            )

                def finish_a(ic):
                    # normalize (per-partition scalar) + XBAR transpose to oT
                    po, de = pso[ic]
                    rcp = ab.tile([128, 2, 4, 1], F32, tag="rcp", bufs=2, name="rcp")
                    nc.vector.reciprocal(rcp[:], de[:])
                    for isub in range(2):
                        on = ab.tile([128, 4, 64], BF16, tag="onat", bufs=2, name="on")
                        nc.vector.tensor_tensor(
                            on[:], po[:, isub, :, :],
                            rcp[:, isub, :, :].to_broadcast((128, 4, 64)), AL.mult,
                        )
                        for cc in range(2):
                            nc.sync.dma_start_transpose(
                                oT[:, cc, ic, isub * 128:(isub + 1) * 128],
                                on[:, 2 * cc:2 * cc + 2, :].rearrange(
                                    "p a b -> p (a b)"
                                ),
                            )
                    del pso[ic]

                # rotate the pss slot assignment so the first scores land in
                # the bank pair whose front-phase user finished earliest
                ap.tile([128, 4, ICS], F32, tag="pss", bufs=3, name="pssrot")
                NG = NIC * NJC
                LK = 4  # PV lookahead in jc-steps
                for g in range(NG + LK):
                    if g < NG:
                        scores_exp(g)
                    if g >= LK:
                        pv(g - LK)
                        r = g - LK
                        if r % NJC == NJC - 1:
                            finish_a(r // NJC)

            # ---- out_proj tail: oT gathered for all ics ---------------
            with (
                tc.tile_pool(name="tsb", bufs=1) as tb,
                tc.tile_pool(name="tps", bufs=1, space="PSUM") as tp,
            ):
                for ic in range(NIC):
                    for isub in range(2):
                        for ec in range(2):
                            psy = tp.tile([128, 512], F32, tag="psy", bufs=4,
                                          name="psy")
                            for cc in range(2):
                                nc.tensor.matmul(
                                    psy[:],
                                    lhsT=oT[:, cc, ic,
                                            isub * 128:(isub + 1) * 128],
                                    rhs=wo_sb[:, cc, ec * 512:(ec + 1) * 512],
                                    start=(cc == 0), stop=(cc == 1),
                                )
                            dst = y_sb[:, ic, isub, ec * 512:(ec + 1) * 512]
                            if ec == 0:
                                nc.vector.tensor_copy(dst, psy[:])
                            else:
                                nc.scalar.copy(dst, psy[:])
                        nc.sync.dma_start(
                            out=y[:, ic, isub, :], in_=y_sb[:, ic, isub, :]
                        )

    nc.compile()
    return nc


def _get(use_mask: bool, use_bias: bool):
    key = (use_mask, use_bias)
    if key not in _cache:
        _cache[key] = _build(use_mask, use_bias)
    return _cache[key]


def _host_prep(hidden_state, attention_mask, Wq, bq, Wk, bk, Wv, bv, Wo,
               use_mask, use_bias):
    half_q, half_k = HID // 2, (HKV * D) // 2  # 512, 128
    inv_q = ROPE_BASE ** (-np.arange(half_q, dtype=np.float64) / half_q)
    inv_k = ROPE_BASE ** (-np.arange(half_k, dtype=np.float64) / half_k)
    s_idx = np.arange(S, dtype=np.float64)
    cos_q = np.cos(inv_q[:, None] * s_idx[None, :])  # [512, S]
    sin_q = np.sin(inv_q[:, None] * s_idx[None, :])
    cos_k = np.cos(inv_k[:, None] * s_idx[None, :])  # [128, S]
    sin_k = np.sin(inv_k[:, None] * s_idx[None, :])

    in_maps = []
    for core in range(8):
        b, s = core // 4, core % 4
        qA = np.arange(128 * s, 128 * s + 128)       # chA q cols
        qB = qA + 512                                 # chB q cols
        kva = s // 2
        kA = np.arange(64 * kva, 64 * kva + 64)       # kv_a cols
        kB = kA + 128                                 # kv_b cols

        # hT layout: [hidden-dim-within-chunk, ko-chunk, S]
        hTc = np.ascontiguousarray(
            hidden_state[b].T.reshape(8, 128, S).transpose(1, 0, 2)
        ).astype(NB)
        wq_c = np.stack(
            [Wq[:, np.concatenate([qA, qB])][ko * 128:(ko + 1) * 128]
             for ko in range(8)], axis=1,
        ).astype(NB)  # [128, 8, 256]
        wk_cols = np.concatenate([Wk[:, kA], Wk[:, kB]], axis=1)  # [HID, 128]
        wv_cols = np.concatenate([Wv[:, kA], Wv[:, kB]], axis=1)  # [HID, 128]
        wkv_c = np.stack(
            [np.concatenate([wk_cols, wv_cols], axis=1)[ko * 128:(ko + 1) * 128]
             for ko in range(8)], axis=1,
        ).astype(NB)  # [128, 8, 256]
        worows = np.concatenate([qA[0:64], qB[0:64], qA[64:128], qB[64:128]])
        wo_c = Wo[worows].astype(NB).reshape(2, 128, HID).transpose(1, 0, 2)
        wo_c = np.ascontiguousarray(wo_c)
        qtab_c = np.stack(
            [cos_q[qA % 512], sin_q[qA % 512]], axis=1
        ).astype(NB)  # [128, 2, S]
        ktab_c = np.stack(
            [cos_k[kA % 128], sin_k[kA % 128]], axis=1
        ).astype(NB)  # [64, 2, S]

        m = {
            "hT": hTc, "wq": wq_c, "wkv": wkv_c, "wo": wo_c,
            "qtab": qtab_c, "ktab": ktab_c,
        }
        if use_bias:
            m["bias"] = np.concatenate(
                [bq[qA], bq[qB], bk[kA], bk[kB], bv[kA], bv[kB]]
            ).astype(np.float32).reshape(1, 512)
        if use_mask:
            mT = np.ascontiguousarray(attention_mask[b].T).astype(np.float32)
            m["mk"] = mT.reshape(NJC, 128, S)
        in_maps.append(m)
    return in_maps


def kernel(hidden_state, attention_mask, Wq, bq, Wk, bk, Wv, bv, Wo, bo):
    hidden_state = np.asarray(hidden_state, dtype=np.float32)
    attention_mask = np.asarray(attention_mask, dtype=np.float32)
    bq, bk, bv = (np.asarray(x, np.float32) for x in (bq, bk, bv))
    use_mask = bool(np.any(attention_mask))
    use_bias = bool(np.any(bq) or np.any(bk) or np.any(bv))
    nc = _get(use_mask, use_bias)
    in_maps = _host_prep(
        hidden_state, attention_mask,
        np.asarray(Wq, np.float32), bq,
        np.asarray(Wk, np.float32), bk,
        np.asarray(Wv, np.float32), bv,
        np.asarray(Wo, np.float32), use_mask, use_bias,
    )
    res = run_bass_kernel_spmd(nc, in_maps, list(range(8)))
    out = np.zeros((B, S, HID), dtype=np.float32)
    for core in range(8):
        yc = res.results[core]["y"].astype(np.float32)  # [128, NIC, 2, HID]
        out[core // 4] += yc.transpose(1, 2, 0, 3).reshape(S, HID)
    out += np.asarray(bo, np.float32)[None, None, :]
    return out

